# revision 5
# baseline (speedup 1.0000x reference)
"""MoE MLP (GPT-2 style experts, top-2 routing) on 8 Trainium2 NeuronCores.

Strategy (expert-parallel, per sharding hint):
  - Host: router matmul + softmax + top-2 + renormalize (tiny: N x 1024 @ 1024 x 8).
  - Host: dispatch tokens by expert id -> per-core gathered token block,
    transposed to [C, M] so the device kernel only does natural-layout matmuls.
  - Device (core e), per m-chunk pair (a, b <= 896 tokens total):
      fc:   hT = gelu(w_fc[e].T @ xT + b_fc[e])   [ff, m] layout
            - stationary = w_fc 128x128 block, shared by the pair's two
              moving matmuls -> walrus --enable-ldw-opt elides the second
              LDWEIGHTS (redundant load-weight optimization).
            - one 2-bank PSUM tile [128, a+b], single Gelu ACT per f-block.
      proj: out[m, c] = w_proj[e].T-contract over ff, activation-stationary:
            stationary = hT[ffb, mb(128 tokens)], moving = w_proj[ffb, 0:1024]
            split as 2x512 psum chunks -> again 2 matmuls per LDWEIGHTS.
            Output is token-major [m, c] fp16, DMA'd straight out.
  - Host: combine: out[tok] += gate * (y + b_proj[e]) for each routed pair.

All matmuls run fp16 in DoublePixel perf mode (2 moving pixels/cycle,
bit-identical results) with f32 PSUM accumulation. Both weight matrices
stay resident in SBUF (fp16): 64+64 KB/partition, hT pair buffer 56 KB.
"""

import functools
import os

import numpy as np

import concourse.bacc as bacc
import concourse.mybir as mybir
import concourse.tile as tile
from concourse.bass_utils import run_bass_kernel_spmd

N_EMBD = 1024
D_FF = 4096
N_EXPERTS = 8
TOP_K = 2
N_CORES = 8
P = 128
KT = N_EMBD // P      # 8 k-tiles (contraction over n_embd)
FT = D_FF // P        # 32 ff-tiles
CT = N_EMBD // P      # 8 output-channel tiles
PAIR = 896            # tokens per fc pair (2 PSUM banks, 56KB/part hT)

DT16 = mybir.dt.float16
F32 = mybir.dt.float32
DP = mybir.MatmulPerfMode.DoublePixel


def _clean(si):
    return si is None or (
        not getattr(si, "waits", None) and not getattr(si, "updates", None)
    )


def _dedupe_ldweights(fn):
    """Drop InstLdweights whose stationary AP equals the previous LDWEIGHTS
    in the PE stream (the paired matmuls deliberately share a stationary).
    Only sem-free LDWs are dropped, so semaphore semantics are unchanged.
    Then verify: every matmul's stationary operand must match the weights
    last loaded into the PE array at its position in the stream."""
    removed = 0
    for bb in fn.blocks:
        insts = bb.instructions
        keep = []
        last = None
        changed = False
        for inst in insts:
            tn = type(inst).__name__
            if tn == "InstLdweights":
                k = str(inst.ins[0])
                if last == k and _clean(inst.sync_info):
                    removed += 1
                    changed = True
                    continue
                last = k
            keep.append(inst)
        if changed:
            insts[:] = keep
    loaded = None
    for bb in fn.blocks:
        for inst in bb.instructions:
            tn = type(inst).__name__
            if tn == "InstLdweights":
                loaded = str(inst.ins[0])
            elif tn == "InstMatmult":
                assert loaded == str(inst.ins[1]), (
                    f"LDW dedupe broke weight pairing at {inst.name}"
                )
    return removed


def _m_pairs(M):
    """Split M (multiple of 128) into pairs (a, b), a+b <= PAIR, each a
    multiple of 128; b may be 0 for the tail."""
    out = []
    pos = 0
    while pos < M:
        a = min(512, M - pos)
        b = min(PAIR - 512, M - pos - a)
        out.append((pos, a, b))
        pos += a + b
    return out


@functools.lru_cache(maxsize=8)
def _build(M, repeat=1):
    """Bass program: per-core dense expert MLP over M gathered tokens."""
    nc = bacc.Bacc("TRN2", target_bir_lowering=False, debug=False)

    xT = nc.dram_tensor("xT", [KT, P, M], DT16, kind="ExternalInput")
    wfc = nc.dram_tensor("w_fc", [KT, P, D_FF], DT16, kind="ExternalInput")
    bfcT = nc.dram_tensor("b_fcT", [P, FT], F32, kind="ExternalInput")
    wproj = nc.dram_tensor("w_proj", [FT, P, N_EMBD], DT16, kind="ExternalInput")
    out = nc.dram_tensor("out", [M, N_EMBD], DT16, kind="ExternalOutput")

    pairs = _m_pairs(M)

    with tile.TileContext(nc) as tc:
        with tc.tile_pool(name="weights", bufs=1) as wpool, \
             tc.tile_pool(name="xp", bufs=1) as xpool, \
             tc.tile_pool(name="hp", bufs=1) as hpool, \
             tc.tile_pool(name="op", bufs=2) as opool, \
             tc.tile_pool(name="psA", bufs=2, space="PSUM") as psA, \
             tc.tile_pool(name="psB", bufs=2, space="PSUM") as psB:

            def load_x(m0, mw):
                x_sb = xpool.tile([P, KT, PAIR], DT16, tag="x", name="x_sb")
                for k in range(KT):
                    nc.sync.dma_start(x_sb[:, k, :mw], xT[k, :, m0:m0 + mw])
                return x_sb

            # First pair's tokens queued ahead of the weights so the PE can
            # start as soon as the first w_fc column-chunk lands.
            m0_0, a_0, b_0 = pairs[0]
            pre_x = load_x(m0_0, a_0 + b_0)

            wfc_sb = wpool.tile([P, KT, D_FF], DT16, tag="wfc", name="wfc_sb")
            CHUNK = 1024
            for c0 in range(0, D_FF, CHUNK):
                for k in range(KT):
                    nc.sync.dma_start(
                        wfc_sb[:, k, c0:c0 + CHUNK],
                        wfc[k, :, c0:c0 + CHUNK]
                    )
            bfc_sb = wpool.tile([P, FT], F32, tag="bfc", name="bfc_sb")
            nc.sync.dma_start(bfc_sb[:, :], bfcT[:, :])
            wproj_sb = wpool.tile([P, FT, N_EMBD], DT16, tag="wproj",
                                  name="wproj_sb")
            for f in range(FT):
                nc.sync.dma_start(wproj_sb[:, f, :], wproj[f, :, :])

            for _r in range(repeat):
                for pi, (m0, a, b) in enumerate(pairs):
                    mw = a + b
                    x_sb = pre_x if (_r == 0 and pi == 0) else load_x(m0, mw)

                    # --- fc: hT[f, m] = gelu(wfc.T @ x + b) ---------------
                    hT_sb = hpool.tile([P, FT, PAIR], DT16, tag="h")
                    for f in range(FT):
                        ps = psA.tile([P, PAIR], F32, tag="psA")
                        for k in range(KT):
                            w_ap = wfc_sb[:, k, f * P:(f + 1) * P]
                            nc.tensor.matmul(
                                ps[:, :a], w_ap, x_sb[:, k, :a],
                                start=(k == 0), stop=(k == KT - 1),
                                perf_mode=DP,
                            )
                            if b:
                                nc.tensor.matmul(
                                    ps[:, a:mw], w_ap, x_sb[:, k, a:mw],
                                    start=(k == 0), stop=(k == KT - 1),
                                    perf_mode=DP,
                                )
                        nc.scalar.activation(
                            hT_sb[:, f, :mw], ps[:, :mw],
                            mybir.ActivationFunctionType.Gelu,
                            bias=bfc_sb[:, f:f + 1],
                        )

                    # --- proj: out[m, c], activation-stationary ----------
                    for mb in range(mw // P):
                        ps2 = psB.tile([P, N_EMBD], F32, tag="psB")
                        for f in range(FT):
                            h_ap = hT_sb[:, f, mb * P:(mb + 1) * P]
                            nc.tensor.matmul(
                                ps2[:, 0:512], h_ap,
                                wproj_sb[:, f, 0:512],
                                start=(f == 0), stop=(f == FT - 1),
                                perf_mode=DP,
                            )
                            nc.tensor.matmul(
                                ps2[:, 512:1024], h_ap,
                                wproj_sb[:, f, 512:1024],
                                start=(f == 0), stop=(f == FT - 1),
                                perf_mode=DP,
                            )
                        o_sb = opool.tile([P, N_EMBD], DT16, tag="o")
                        nc.vector.tensor_copy(o_sb[:, :], ps2[:, :])
                        nc.sync.dma_start(
                            out[m0 + mb * P:m0 + (mb + 1) * P, :], o_sb[:, :]
                        )

    nc.compile()
    _dedupe_ldweights(nc.m.functions[0])
    return nc


def _route(x_flat, router_w):
    """Top-2 routing, matching the reference numerics (f32)."""
    N = x_flat.shape[0]
    logits = x_flat @ router_w.T                      # [N, E]
    logits -= logits.max(axis=-1, keepdims=True)
    p = np.exp(logits)
    p /= p.sum(axis=-1, keepdims=True)
    rows = np.arange(N)
    i1 = p.argmax(axis=-1)
    p1 = p[rows, i1]
    pm = p.copy()
    pm[rows, i1] = -1.0
    i2 = pm.argmax(axis=-1)
    p2 = p[rows, i2]
    s = p1 + p2 + 1e-9
    return i1, i2, p1 / s, p2 / s


def prepare_in_maps(x_flat, idxs, w_fc, b_fc, w_proj, M):
    """Per-core input dict for _build's dram tensors."""
    in_maps = []
    for e in range(N_EXPERTS):
        idx = idxs[e]
        xg = np.zeros((M, N_EMBD), dtype=np.float32)
        xg[: len(idx)] = x_flat[idx]
        xT = np.ascontiguousarray(xg.T).reshape(KT, P, M).astype(np.float16)
        in_maps.append({
            "xT": xT,
            "w_fc": w_fc[e].reshape(KT, P, D_FF).astype(np.float16),
            "b_fcT": np.ascontiguousarray(b_fc[e].reshape(FT, P).T),
            "w_proj": w_proj[e].reshape(FT, P, N_EMBD).astype(np.float16),
        })
    return in_maps


def kernel(x, router_w, w_fc, b_fc, w_proj, b_proj):
    x = np.asarray(x, dtype=np.float32)
    router_w = np.asarray(router_w, dtype=np.float32)
    w_fc = np.asarray(w_fc, dtype=np.float32)
    b_fc = np.asarray(b_fc, dtype=np.float32)
    w_proj = np.asarray(w_proj, dtype=np.float32)
    b_proj = np.asarray(b_proj, dtype=np.float32)

    B, T, C = x.shape
    x_flat = x.reshape(-1, C)
    N = x_flat.shape[0]

    i1, i2, g1, g2 = _route(x_flat, router_w)

    idxs, gates = [], []
    for e in range(N_EXPERTS):
        mask = (i1 == e) | (i2 == e)
        idx = np.flatnonzero(mask)
        g = np.where(i1[idx] == e, g1[idx], g2[idx]).astype(np.float32)
        idxs.append(idx)
        gates.append(g)

    max_cnt = max(len(ix) for ix in idxs)
    M = max(P, ((max_cnt + P - 1) // P) * P)

    repeat = int(os.environ.get("MOE_KERNEL_REPEAT", "1"))
    nc = _build(M, repeat)

    in_maps = prepare_in_maps(x_flat, idxs, w_fc, b_fc, w_proj, M)
    res = run_bass_kernel_spmd(nc, in_maps, core_ids=list(range(N_CORES)))

    out_flat = np.zeros((N, C), dtype=np.float32)
    for e in range(N_EXPERTS):
        idx = idxs[e]
        y = res.results[e]["out"][: len(idx)].astype(np.float32)  # [n_e, C]
        out_flat[idx] += gates[e][:, None] * (y + b_proj[e])

    return out_flat.reshape(B, T, C)


# revision 6
# speedup vs baseline: 1.5224x; 1.5224x over previous
"""MoE MLP (GPT-2 style experts, top-2 routing) on 8 Trainium2 NeuronCores.

Strategy (expert-parallel, per sharding hint):
  - Host: router matmul + softmax + top-2 + renormalize (tiny: N x 1024 @ 1024 x 8).
  - Host: dispatch tokens by expert id -> per-core gathered token block,
    transposed to [C, M] so the device kernel only does natural-layout matmuls.
  - Device (core e), per m-chunk pair (a, b <= 896 tokens total):
      fc:   hT = gelu(w_fc[e].T @ xT + b_fc[e])   [ff, m] layout
            - stationary = w_fc 128x128 block, shared by the pair's two
              moving matmuls -> walrus --enable-ldw-opt elides the second
              LDWEIGHTS (redundant load-weight optimization).
            - one 2-bank PSUM tile [128, a+b], single Gelu ACT per f-block.
      proj: out[m, c] = w_proj[e].T-contract over ff, activation-stationary:
            stationary = hT[ffb, mb(128 tokens)], moving = w_proj[ffb, 0:1024]
            split as 2x512 psum chunks -> again 2 matmuls per LDWEIGHTS.
            Output is token-major [m, c] fp16, DMA'd straight out.
  - Host: combine: out[tok] += gate * (y + b_proj[e]) for each routed pair.

All matmuls run fp16 in DoublePixel perf mode (2 moving pixels/cycle,
bit-identical results) with f32 PSUM accumulation. Both weight matrices
stay resident in SBUF (fp16): 64+64 KB/partition, hT pair buffer 56 KB.
"""

import functools
import os

import numpy as np

import concourse.bacc as bacc
import concourse.mybir as mybir
import concourse.tile as tile
from concourse.bass_utils import run_bass_kernel_spmd

N_EMBD = 1024
D_FF = 4096
N_EXPERTS = 8
TOP_K = 2
N_CORES = 8
P = 128
KT = N_EMBD // P      # 8 k-tiles (contraction over n_embd)
FT = D_FF // P        # 32 ff-tiles
CT = N_EMBD // P      # 8 output-channel tiles
PAIR = 896            # tokens per fc pair (2 PSUM banks, 56KB/part hT)

DT16 = mybir.dt.float16
F32 = mybir.dt.float32
DP = mybir.MatmulPerfMode.DoublePixel


def _clean(si):
    return si is None or (
        not getattr(si, "waits", None) and not getattr(si, "updates", None)
    )


def _dedupe_ldweights(fn):
    """Drop InstLdweights whose stationary AP equals the previous LDWEIGHTS
    in the PE stream (the paired matmuls deliberately share a stationary).
    Only sem-free LDWs are dropped, so semaphore semantics are unchanged.
    Then verify: every matmul's stationary operand must match the weights
    last loaded into the PE array at its position in the stream."""
    removed = 0
    for bb in fn.blocks:
        insts = bb.instructions
        keep = []
        last = None
        changed = False
        for inst in insts:
            tn = type(inst).__name__
            if tn == "InstLdweights":
                k = str(inst.ins[0])
                if last == k and _clean(inst.sync_info):
                    removed += 1
                    changed = True
                    continue
                last = k
            keep.append(inst)
        if changed:
            insts[:] = keep
    loaded = None
    for bb in fn.blocks:
        for inst in bb.instructions:
            tn = type(inst).__name__
            if tn == "InstLdweights":
                loaded = str(inst.ins[0])
            elif tn == "InstMatmult":
                assert loaded == str(inst.ins[1]), (
                    f"LDW dedupe broke weight pairing at {inst.name}"
                )
    return removed


def _m_pairs(M):
    """Split M (multiple of 128) into pairs (a, b), a+b <= PAIR, each a
    multiple of 128; b may be 0 for the tail."""
    out = []
    pos = 0
    while pos < M:
        a = min(512, M - pos)
        b = min(PAIR - 512, M - pos - a)
        out.append((pos, a, b))
        pos += a + b
    return out


@functools.lru_cache(maxsize=8)
def _build(M, repeat=1):
    """Bass program: per-core dense expert MLP over M gathered tokens."""
    nc = bacc.Bacc("TRN2", target_bir_lowering=False, debug=False)

    xT = nc.dram_tensor("xT", [KT, P, M], DT16, kind="ExternalInput")
    wfc = nc.dram_tensor("w_fc", [KT, P, D_FF], DT16, kind="ExternalInput")
    bfcT = nc.dram_tensor("b_fcT", [P, FT], F32, kind="ExternalInput")
    wproj = nc.dram_tensor("w_proj", [FT, P, N_EMBD], DT16, kind="ExternalInput")
    out = nc.dram_tensor("out", [M, N_EMBD], DT16, kind="ExternalOutput")

    pairs = _m_pairs(M)

    with tile.TileContext(nc) as tc:
        with tc.tile_pool(name="weights", bufs=1) as wpool, \
             tc.tile_pool(name="xp", bufs=1) as xpool, \
             tc.tile_pool(name="hp", bufs=1) as hpool, \
             tc.tile_pool(name="op", bufs=2) as opool, \
             tc.tile_pool(name="psA", bufs=2, space="PSUM") as psA, \
             tc.tile_pool(name="psB", bufs=2, space="PSUM") as psB:

            def load_x(m0, mw):
                x_sb = xpool.tile([P, KT, PAIR], DT16, tag="x", name="x_sb")
                for k in range(KT):
                    nc.sync.dma_start(x_sb[:, k, :mw], xT[k, :, m0:m0 + mw])
                return x_sb

            # First pair's tokens queued ahead of the weights so the PE can
            # start as soon as the first w_fc column-chunk lands.
            m0_0, a_0, b_0 = pairs[0]
            pre_x = load_x(m0_0, a_0 + b_0)

            wfc_sb = wpool.tile([P, KT, D_FF], DT16, tag="wfc", name="wfc_sb")
            CHUNK = 1024
            for c0 in range(0, D_FF, CHUNK):
                for k in range(KT):
                    nc.sync.dma_start(
                        wfc_sb[:, k, c0:c0 + CHUNK],
                        wfc[k, :, c0:c0 + CHUNK]
                    )
            bfc_sb = wpool.tile([P, FT], F32, tag="bfc", name="bfc_sb")
            nc.sync.dma_start(bfc_sb[:, :], bfcT[:, :])
            wproj_sb = wpool.tile([P, FT, N_EMBD], DT16, tag="wproj",
                                  name="wproj_sb")
            for f in range(FT):
                nc.sync.dma_start(wproj_sb[:, f, :], wproj[f, :, :])

            for _r in range(repeat):
                for pi, (m0, a, b) in enumerate(pairs):
                    mw = a + b
                    x_sb = pre_x if (_r == 0 and pi == 0) else load_x(m0, mw)

                    # --- fc: hT[f, m] = gelu(wfc.T @ x + b) ---------------
                    hT_sb = hpool.tile([P, FT, PAIR], DT16, tag="h")
                    for f in range(FT):
                        ps = psA.tile([P, PAIR], F32, tag="psA")
                        for k in range(KT):
                            w_ap = wfc_sb[:, k, f * P:(f + 1) * P]
                            nc.tensor.matmul(
                                ps[:, :a], w_ap, x_sb[:, k, :a],
                                start=(k == 0), stop=(k == KT - 1),
                                perf_mode=DP,
                            )
                            if b:
                                nc.tensor.matmul(
                                    ps[:, a:mw], w_ap, x_sb[:, k, a:mw],
                                    start=(k == 0), stop=(k == KT - 1),
                                    perf_mode=DP,
                                )
                        nc.scalar.activation(
                            hT_sb[:, f, :mw], ps[:, :mw],
                            mybir.ActivationFunctionType.Gelu,
                            bias=bfc_sb[:, f:f + 1],
                        )

                    # --- proj: out[m, c], activation-stationary ----------
                    for mb in range(mw // P):
                        ps2 = psB.tile([P, N_EMBD], F32, tag="psB")
                        for f in range(FT):
                            h_ap = hT_sb[:, f, mb * P:(mb + 1) * P]
                            nc.tensor.matmul(
                                ps2[:, 0:512], h_ap,
                                wproj_sb[:, f, 0:512],
                                start=(f == 0), stop=(f == FT - 1),
                                perf_mode=DP,
                            )
                            nc.tensor.matmul(
                                ps2[:, 512:1024], h_ap,
                                wproj_sb[:, f, 512:1024],
                                start=(f == 0), stop=(f == FT - 1),
                                perf_mode=DP,
                            )
                        o_sb = opool.tile([P, N_EMBD], DT16, tag="o")
                        nc.vector.tensor_copy(o_sb[:, :], ps2[:, :])
                        nc.sync.dma_start(
                            out[m0 + mb * P:m0 + (mb + 1) * P, :], o_sb[:, :]
                        )

    nc.compile()
    if os.environ.get("MOE_NO_DEDUPE") != "1":
        _dedupe_ldweights(nc.m.functions[0])
    return nc


def _route(x_flat, router_w):
    """Top-2 routing, matching the reference numerics (f32)."""
    N = x_flat.shape[0]
    logits = x_flat @ router_w.T                      # [N, E]
    logits -= logits.max(axis=-1, keepdims=True)
    p = np.exp(logits)
    p /= p.sum(axis=-1, keepdims=True)
    rows = np.arange(N)
    i1 = p.argmax(axis=-1)
    p1 = p[rows, i1]
    pm = p.copy()
    pm[rows, i1] = -1.0
    i2 = pm.argmax(axis=-1)
    p2 = p[rows, i2]
    s = p1 + p2 + 1e-9
    return i1, i2, p1 / s, p2 / s


def prepare_in_maps(x_flat, idxs, w_fc, b_fc, w_proj, M):
    """Per-core input dict for _build's dram tensors."""
    in_maps = []
    for e in range(N_EXPERTS):
        idx = idxs[e]
        xg = np.zeros((M, N_EMBD), dtype=np.float32)
        xg[: len(idx)] = x_flat[idx]
        xT = np.ascontiguousarray(xg.T).reshape(KT, P, M).astype(np.float16)
        in_maps.append({
            "xT": xT,
            "w_fc": w_fc[e].reshape(KT, P, D_FF).astype(np.float16),
            "b_fcT": np.ascontiguousarray(b_fc[e].reshape(FT, P).T),
            "w_proj": w_proj[e].reshape(FT, P, N_EMBD).astype(np.float16),
        })
    return in_maps


def kernel(x, router_w, w_fc, b_fc, w_proj, b_proj):
    x = np.asarray(x, dtype=np.float32)
    router_w = np.asarray(router_w, dtype=np.float32)
    w_fc = np.asarray(w_fc, dtype=np.float32)
    b_fc = np.asarray(b_fc, dtype=np.float32)
    w_proj = np.asarray(w_proj, dtype=np.float32)
    b_proj = np.asarray(b_proj, dtype=np.float32)

    B, T, C = x.shape
    x_flat = x.reshape(-1, C)
    N = x_flat.shape[0]

    i1, i2, g1, g2 = _route(x_flat, router_w)

    idxs, gates = [], []
    for e in range(N_EXPERTS):
        mask = (i1 == e) | (i2 == e)
        idx = np.flatnonzero(mask)
        g = np.where(i1[idx] == e, g1[idx], g2[idx]).astype(np.float32)
        idxs.append(idx)
        gates.append(g)

    max_cnt = max(len(ix) for ix in idxs)
    M = max(P, ((max_cnt + P - 1) // P) * P)

    repeat = int(os.environ.get("MOE_KERNEL_REPEAT", "1"))
    nc = _build(M, repeat)

    in_maps = prepare_in_maps(x_flat, idxs, w_fc, b_fc, w_proj, M)
    res = run_bass_kernel_spmd(nc, in_maps, core_ids=list(range(N_CORES)))

    out_flat = np.zeros((N, C), dtype=np.float32)
    for e in range(N_EXPERTS):
        idx = idxs[e]
        y = res.results[e]["out"][: len(idx)].astype(np.float32)  # [n_e, C]
        out_flat[idx] += gates[e][:, None] * (y + b_proj[e])

    return out_flat.reshape(B, T, C)
